# revision 6
# baseline (speedup 1.0000x reference)
"""DIN (attention pooling + MLP w/ BatchNorm+Dice) Trainium2 kernel, 8-core SPMD.

Contract: kernel(**inputs) takes FULL unsharded inputs (numpy), returns [4096,1] f32.
Internally: batch-sharded 512 samples/core; embedding tables + weights replicated.

v2: batched indirect gathers (1 call/tile), bf16 datapath, DMA-transposes,
L1 matmul accumulated in PSUM during attention, log-tree reductions on DVE.
"""
import sys
sys.path.insert(0, "/opt/trn_rl_repo")
import numpy as np
import ml_dtypes

import concourse.bass as bass
import concourse.mybir as mybir
import concourse.tile as tile
from concourse import bacc
from concourse.bass_utils import run_bass_kernel_spmd

B, H, D = 4096, 100, 128
N_ITEM, M_USER = 100000, 500000
D1, D2 = 1024, 512
NCORES = 8
BL = B // NCORES          # 512 samples per core
T = BL // 128             # 4 sample-tiles per core
DICE_EPS, BN_EPS = 1e-3, 1e-5

BF = mybir.dt.bfloat16
F32 = mybir.dt.float32
I32 = mybir.dt.int32
MUL = mybir.AluOpType.mult
ADD = mybir.AluOpType.add
SUB = mybir.AluOpType.subtract
AF = mybir.ActivationFunctionType
X_AX = mybir.AxisListType.X

_PROG = None


def _bcol(col_ap, n):
    """[128,1] AP -> [128,n] broadcast AP (step-0 inner dim)."""
    return col_ap.to_broadcast([col_ap.shape[0], n])


def _build(sim_mode=False):
    ndev = 1 if sim_mode else NCORES
    nc = bacc.Bacc("TRN2", target_bir_lowering=False, debug=False, num_devices=ndev)

    user_tab = nc.dram_tensor("user_table", [M_USER, D], F32, kind="ExternalInput")
    item_tab = nc.dram_tensor("item_table", [N_ITEM, D], F32, kind="ExternalInput")
    idx_hist = nc.dram_tensor("idx_hist", [128, T * H], I32, kind="ExternalInput")
    idx_ui = nc.dram_tensor("idx_ui", [128, 2 * T], I32, kind="ExternalInput")
    w1d = nc.dram_tensor("w1sb", [128, 3 * D1], BF, kind="ExternalInput")
    w2d = nc.dram_tensor("w2sb", [128, 8 * D2], BF, kind="ExternalInput")
    w3d = nc.dram_tensor("w3sb", [128, 4], BF, kind="ExternalInput")
    g1d = nc.dram_tensor("g1r", [128, 8], F32, kind="ExternalInput")
    be1d = nc.dram_tensor("be1r", [128, 8], F32, kind="ExternalInput")
    g2d = nc.dram_tensor("g2r", [128, 4], F32, kind="ExternalInput")
    be2d = nc.dram_tensor("be2r", [128, 4], F32, kind="ExternalInput")
    a1d = nc.dram_tensor("a1c", [128, 1], F32, kind="ExternalInput")
    a2d = nc.dram_tensor("a2c", [128, 1], F32, kind="ExternalInput")
    b3d = nc.dram_tensor("b3c", [1, 1], F32, kind="ExternalInput")
    outd = nc.dram_tensor("out", [1, BL], F32, kind="ExternalOutput")

    with tile.TileContext(nc) as tc:
        with (
            tc.tile_pool(name="sb", bufs=1) as sb,
            tc.tile_pool(name="rot", bufs=2) as rot,
            tc.tile_pool(name="dram", bufs=1, space="DRAM") as dr,
        ):
            # ---------- index / weight uploads ----------
            idxh = sb.tile([128, T * H], I32)
            nc.sync.dma_start(out=idxh[:], in_=idx_hist[:])
            idxui = sb.tile([128, 2 * T], I32)
            nc.sync.dma_start(out=idxui[:], in_=idx_ui[:])
            w1 = sb.tile([128, 3 * D1], BF)
            nc.sync.dma_start(out=w1[:], in_=w1d[:])
            w2 = sb.tile([128, 8 * D2], BF)
            nc.sync.dma_start(out=w2[:], in_=w2d[:])
            w3 = sb.tile([128, 4], BF)
            nc.sync.dma_start(out=w3[:], in_=w3d[:])
            g1 = sb.tile([128, 8], F32)
            nc.sync.dma_start(out=g1[:], in_=g1d[:])
            be1 = sb.tile([128, 8], F32)
            nc.sync.dma_start(out=be1[:], in_=be1d[:])
            g2 = sb.tile([128, 4], F32)
            nc.sync.dma_start(out=g2[:], in_=g2d[:])
            be2 = sb.tile([128, 4], F32)
            nc.sync.dma_start(out=be2[:], in_=be2d[:])
            a1s = sb.tile([128, 1], F32)
            nc.sync.dma_start(out=a1s[:], in_=a1d[:])
            a2s = sb.tile([128, 1], F32)
            nc.sync.dma_start(out=a2s[:], in_=a2d[:])
            b3s = sb.tile([1, 1], F32)
            nc.sync.dma_start(out=b3s[:], in_=b3d[:])

            ones_bf = sb.tile([128, 1], BF)       # 1.0
            nc.gpsimd.memset(ones_bf[:], 1.0)
            ones_d1 = sb.tile([128, 1], BF)       # 1/1024
            nc.gpsimd.memset(ones_d1[:], 1.0 / D1)
            ones_d2 = sb.tile([128, 1], BF)       # 1/512
            nc.gpsimd.memset(ones_d2[:], 1.0 / D2)
            onesrow_bf = sb.tile([1, 128], BF)
            nc.gpsimd.memset(onesrow_bf[:], 1.0)
            eps_bn = sb.tile([128, 1], F32)
            nc.gpsimd.memset(eps_bn[:], BN_EPS)
            epsd1_row = sb.tile([1, 1], F32)
            nc.gpsimd.memset(epsd1_row[:], DICE_EPS * D1)
            epsd2_row = sb.tile([1, 1], F32)
            nc.gpsimd.memset(epsd2_row[:], DICE_EPS * D2)

            # dice alpha scalars
            oma1 = sb.tile([128, 1], F32)  # 1 - a1
            nc.vector.tensor_scalar(out=oma1[:], in0=a1s[:], scalar1=-1.0, scalar2=1.0,
                                    op0=MUL, op1=ADD)
            oma2 = sb.tile([128, 1], F32)
            nc.vector.tensor_scalar(out=oma2[:], in0=a2s[:], scalar1=-1.0, scalar2=1.0,
                                    op0=MUL, op1=ADD)

            # ---------- bulk embedding gathers (1 SWDGE call each) ----------
            # Pool queue order matters: gather t+2 is issued inside the tile
            # loop (after tile-t Pool compute) to avoid WAR stalls on hist bufs.
            hists = []

            def gather_hist(t):
                # indirect DMA honors ONE index per partition per call
                ht = rot.tile([128, H * D], BF, tag="hist", bufs=2, name=f"hist{t}")
                for h in range(H):
                    nc.gpsimd.indirect_dma_start(
                        out=ht[:, h * D:(h + 1) * D], out_offset=None, in_=user_tab[:],
                        in_offset=bass.IndirectOffsetOnAxis(
                            ap=idxh[:, t * H + h:t * H + h + 1], axis=0))
                hists.append(ht)

            uball = sb.tile([128, T * D], BF)
            iball = sb.tile([128, T * D], BF)
            for t in range(T):
                nc.gpsimd.indirect_dma_start(
                    out=uball[:, t * D:(t + 1) * D], out_offset=None, in_=user_tab[:],
                    in_offset=bass.IndirectOffsetOnAxis(ap=idxui[:, t:t + 1], axis=0))
                nc.gpsimd.indirect_dma_start(
                    out=iball[:, t * D:(t + 1) * D], out_offset=None, in_=item_tab[:],
                    in_offset=bass.IndirectOffsetOnAxis(
                        ap=idxui[:, T + t:T + t + 1], axis=0))
            gather_hist(0)
            gather_hist(1)

            xT = sb.tile([128, 3 * BL], BF)       # [feat, 3 k-chunks x 512 samples]
            z1b = sb.tile([128, 8 * D2], BF)      # z1^T drained bf16 (becomes y1)
            statp = sb.tile([128, 16 * T], F32)   # per-(m,t) partial BN1 stats

            HP = 100  # DVE handles h < HP; Pool engine handles h >= HP (Pool busy w/ gathers)

            with tc.tile_pool(name="psA", bufs=1, space="PSUM") as psA:
                zps = [psA.tile([128, D2], F32, tag=f"zp{m}", name=f"zp{m}")
                       for m in range(8)]

                # stage 1 of the software pipeline: products + scores for tile t
                def attn_front(t):
                    hist = hists[t]
                    h3 = hist[:].rearrange("p (h d) -> p h d", d=D)
                    prod = rot.tile([128, HP * D], BF, tag="prod", bufs=1,
                                    name=f"prod{t}")
                    p3 = prod[:].rearrange("p (h d) -> p h d", d=D)
                    scb = rot.tile([128, H * D], BF, tag="scb", bufs=2,
                                   name=f"scb{t}")
                    s3 = scb[:].rearrange("p (h d) -> p h d", d=D)
                    ub = uball[:, t * D:(t + 1) * D].rearrange("p (o d) -> p o d", o=1)
                    # prod1 = hist * user (split DVE / Pool)
                    nc.vector.tensor_tensor(out=p3[:, 0:HP], in0=h3[:, 0:HP],
                                            in1=ub.to_broadcast([128, HP, D]), op=MUL)
                    if HP < H:
                        nc.gpsimd.tensor_tensor(out=s3[:, HP:H], in0=h3[:, HP:H],
                                                in1=ub.to_broadcast([128, H, D])[:, HP:H],
                                                op=MUL)
                    # scores: DVE tree over d for h<HP; Pool reduce for h>=HP
                    for k in (64, 32, 16, 8, 4, 2, 1):
                        nc.vector.tensor_tensor(
                            out=p3[:, :, 0:k], in0=p3[:, :, 0:k],
                            in1=p3[:, :, k:2 * k], op=ADD)
                    if HP < H:
                        nc.gpsimd.tensor_reduce(out=s3[:, HP:H, 0:1], in_=s3[:, HP:H],
                                                axis=X_AX, op=ADD)
                    # compact all scores into scb col 0, then log-double over d:
                    # DVE writes col0 + last level; Act engine does mid levels.
                    nc.vector.tensor_copy(out=s3[:, 0:HP, 0:1], in_=p3[:, 0:HP, 0:1])
                    for k in (1, 2, 4, 8, 16, 32):
                        nc.scalar.activation(out=s3[:, :, k:2 * k],
                                             in_=s3[:, :, 0:k], func=AF.Copy)
                    nc.vector.tensor_copy(out=s3[:, :, 64:128], in_=s3[:, :, 0:64])
                    return scb

                # stage 2: weighted sum + transpose + L1 matmul for tile t
                def attn_back(t, scb):
                    hist = hists[t]
                    h3 = hist[:].rearrange("p (h d) -> p h d", d=D)
                    s3 = scb[:].rearrange("p (h d) -> p h d", d=D)
                    csl = slice(t * 128, (t + 1) * 128)
                    # prod2 = hist * scores, in place over scb
                    nc.vector.tensor_tensor(out=s3[:, 0:HP], in0=h3[:, 0:HP],
                                            in1=s3[:, 0:HP], op=MUL)
                    if HP < H:
                        nc.gpsimd.tensor_tensor(out=s3[:, HP:H], in0=h3[:, HP:H],
                                                in1=s3[:, HP:H], op=MUL)
                    # tree-reduce over h -> user_his emb
                    nc.vector.tensor_tensor(out=s3[:, 0:50], in0=s3[:, 0:50],
                                            in1=s3[:, 50:100], op=ADD)
                    nc.vector.tensor_tensor(out=s3[:, 0:25], in0=s3[:, 0:25],
                                            in1=s3[:, 25:50], op=ADD)
                    nc.vector.tensor_tensor(out=s3[:, 0:12], in0=s3[:, 0:12],
                                            in1=s3[:, 12:24], op=ADD)
                    nc.vector.tensor_tensor(out=s3[:, 0:6], in0=s3[:, 0:6],
                                            in1=s3[:, 6:12], op=ADD)
                    nc.vector.tensor_tensor(out=s3[:, 0:3], in0=s3[:, 0:3],
                                            in1=s3[:, 3:6], op=ADD)
                    nc.vector.tensor_tensor(out=s3[:, 0:1], in0=s3[:, 0:1],
                                            in1=s3[:, 1:2], op=ADD)
                    nc.vector.tensor_tensor(out=s3[:, 0:1], in0=s3[:, 0:1],
                                            in1=s3[:, 2:3], op=ADD)
                    his = rot.tile([128, D], BF, tag="his", bufs=2, name=f"his{t}")
                    nc.vector.tensor_tensor(
                        out=his[:].rearrange("p (o d) -> p o d", o=1),
                        in0=s3[:, 0:1], in1=s3[:, 24:25], op=ADD)
                    nc.sync.dma_start_transpose(
                        out=xT[:, 1 * BL + t * 128:1 * BL + (t + 1) * 128],
                        in_=his[:])

                    # layer-1 matmuls for this tile into resident PSUM + drains
                    for m in range(8):
                        for j, k in enumerate((0, 2, 1)):
                            nc.tensor.matmul(
                                zps[m][:, csl],
                                lhsT=w1[:, k * D1 + m * 128:k * D1 + (m + 1) * 128],
                                rhs=xT[:, k * BL + t * 128:k * BL + (t + 1) * 128],
                                start=(j == 0), stop=(j == 2))
                        nc.scalar.activation(
                            out=z1b[:, m * D2 + t * 128:m * D2 + (t + 1) * 128],
                            in_=zps[m][:, csl], func=AF.Copy,
                            accum_out=statp[:, m * T + t:m * T + t + 1])
                        sqd = rot.tile([128, 128], BF, tag="sqd", bufs=2,
                                       name=f"sqd{t}{m}")
                        nc.scalar.activation(
                            out=sqd[:], in_=zps[m][:, csl], func=AF.Square,
                            accum_out=statp[:, 8 * T + m * T + t:8 * T + m * T + t + 1])

                scbs = {}
                for t in range(T):
                    # item/user transposes into xT (k=0 item, k=2 user)
                    nc.sync.dma_start_transpose(
                        out=xT[:, 0 * BL + t * 128:0 * BL + (t + 1) * 128],
                        in_=iball[:, t * D:(t + 1) * D])
                    nc.sync.dma_start_transpose(
                        out=xT[:, 2 * BL + t * 128:2 * BL + (t + 1) * 128],
                        in_=uball[:, t * D:(t + 1) * D])
                    scbs[t] = attn_front(t)
                    if t + 2 < T:
                        gather_hist(t + 2)
                    if t > 0:
                        attn_back(t - 1, scbs.pop(t - 1))
                attn_back(T - 1, scbs.pop(T - 1))

            # ---------- BN1 stats: merge partials, AllReduce ----------
            stat1 = sb.tile([128, 16], F32)
            nc.vector.tensor_reduce(
                out=stat1[:], in_=statp[:].rearrange("p (s t) -> p s t", t=T),
                axis=X_AX, op=ADD)

            bi1 = dr.tile([128, 16], F32)
            bo1 = dr.tile([128, 16], F32)
            nc.gpsimd.dma_start(out=bi1[:], in_=stat1[:])
            if sim_mode:
                nc.gpsimd.dma_start(out=bo1[:], in_=bi1[:])
            else:
                nc.gpsimd.collective_compute(
                    "AllReduce", ADD, replica_groups=[list(range(NCORES))],
                    ins=[bi1.opt()], outs=[bo1.opt()])
            ast1 = sb.tile([128, 16], F32)
            nc.gpsimd.dma_start(out=ast1[:], in_=bo1[:])

            # BN1 affine: s = g / sqrt(var+eps), t = be - mu*s
            mu1 = sb.tile([128, 8], F32)
            nc.vector.tensor_scalar(out=mu1[:], in0=ast1[:, 0:8], scalar1=1.0 / B,
                                    scalar2=None, op0=MUL)
            var1 = sb.tile([128, 8], F32)
            nc.vector.tensor_scalar(out=var1[:], in0=ast1[:, 8:16], scalar1=1.0 / B,
                                    scalar2=None, op0=MUL)
            musq1 = sb.tile([128, 8], F32)
            nc.vector.tensor_tensor(out=musq1[:], in0=mu1[:], in1=mu1[:], op=MUL)
            nc.vector.tensor_tensor(out=var1[:], in0=var1[:], in1=musq1[:], op=SUB)
            sd1 = sb.tile([128, 8], F32)
            nc.scalar.activation(out=sd1[:], in_=var1[:], func=AF.Sqrt, bias=eps_bn[:])
            inv1 = sb.tile([128, 8], F32)
            nc.vector.reciprocal(out=inv1[:], in_=sd1[:])
            s1 = sb.tile([128, 8], F32)
            nc.vector.tensor_tensor(out=s1[:], in0=g1[:], in1=inv1[:], op=MUL)
            t1 = sb.tile([128, 8], F32)
            nc.vector.tensor_tensor(out=t1[:], in0=mu1[:], in1=s1[:], op=MUL)
            nc.vector.tensor_tensor(out=t1[:], in0=be1[:], in1=t1[:], op=SUB)

            # y1 = s1*z1 + t1 in place (TSP, 4x mode)
            for m in range(8):
                sl = slice(m * D2, (m + 1) * D2)
                nc.vector.tensor_scalar(out=z1b[:, sl], in0=z1b[:, sl],
                                        scalar1=s1[:, m:m + 1], scalar2=t1[:, m:m + 1],
                                        op0=MUL, op1=ADD)
            y1 = z1b

            with tc.tile_pool(name="psB", bufs=1, space="PSUM") as ps:
                # ---------- Dice 1 (feature mean/var via PE ones-matmuls) ----------
                avgp = ps.tile([1, BL], F32, tag="cs", bufs=1)
                for m in range(8):
                    nc.tensor.matmul(avgp[:], lhsT=ones_d1[:], rhs=y1[:, m * D2:(m + 1) * D2],
                                     start=(m == 0), stop=(m == 7))
                avgrow = sb.tile([1, BL], BF)
                nc.vector.tensor_copy(out=avgrow[:], in_=avgp[:])
                avgb = ps.tile([128, BL], F32, tag="bc")
                nc.tensor.matmul(avgb[:], lhsT=onesrow_bf[:], rhs=avgrow[:], start=True, stop=True)
                avgb_sb = sb.tile([128, BL], BF)
                nc.scalar.activation(out=avgb_sb[:], in_=avgb[:], func=AF.Copy)

                diff1 = sb.tile([128, 8 * D2], BF)
                for m in range(8):
                    nc.vector.tensor_tensor(out=diff1[:, m * D2:(m + 1) * D2],
                                            in0=y1[:, m * D2:(m + 1) * D2],
                                            in1=avgb_sb[:], op=SUB)

                varp = ps.tile([1, BL], F32, tag="cs", bufs=1)
                for m in range(8):
                    sq_b = rot.tile([128, D2], BF, tag="sqb")
                    nc.vector.tensor_tensor(out=sq_b[:], in0=diff1[:, m * D2:(m + 1) * D2],
                                            in1=diff1[:, m * D2:(m + 1) * D2], op=MUL)
                    nc.tensor.matmul(varp[:], lhsT=ones_bf[:], rhs=sq_b[:],
                                     start=(m == 0), stop=(m == 7))
                sqrow = sb.tile([1, BL], F32)
                nc.scalar.activation(out=sqrow[:], in_=varp[:], func=AF.Sqrt, bias=epsd1_row[:])
                rstd = sb.tile([1, BL], F32)
                nc.vector.reciprocal(out=rstd[:], in_=sqrow[:])
                rstdb16 = sb.tile([1, BL], BF)
                nc.vector.tensor_copy(out=rstdb16[:], in_=rstd[:])
                rstdb = ps.tile([128, BL], F32, tag="bc")
                nc.tensor.matmul(rstdb[:], lhsT=onesrow_bf[:], rhs=rstdb16[:], start=True, stop=True)
                rstdb_sb = sb.tile([128, BL], BF)
                nc.scalar.activation(out=rstdb_sb[:], in_=rstdb[:], func=AF.Copy)

                for m in range(8):
                    sl = slice(m * D2, (m + 1) * D2)
                    nc.vector.tensor_tensor(out=diff1[:, sl], in0=diff1[:, sl],
                                            in1=rstdb_sb[:], op=MUL)
                    nc.scalar.activation(out=diff1[:, sl], in_=diff1[:, sl], func=AF.Sigmoid)
                    nc.vector.tensor_scalar(out=diff1[:, sl], in0=diff1[:, sl],
                                            scalar1=oma1[:], scalar2=a1s[:], op0=MUL, op1=ADD)
                    nc.vector.tensor_tensor(out=y1[:, sl], in0=y1[:, sl],
                                            in1=diff1[:, sl], op=MUL)

                # ---------- layer 2 (bf16, m-outer with overlapped drains) ----------
                z2b = sb.tile([128, 4 * D2], BF)
                stat2 = sb.tile([128, 8], F32)
                for m in range(4):
                    zp = ps.tile([128, D2], F32, tag="zps", bufs=2)
                    for k in range(8):
                        nc.tensor.matmul(zp[:], lhsT=w2[:, k * D2 + m * 128:k * D2 + (m + 1) * 128],
                                         rhs=y1[:, k * D2:(k + 1) * D2],
                                         start=(k == 0), stop=(k == 7))
                    nc.scalar.activation(out=z2b[:, m * D2:(m + 1) * D2], in_=zp[:], func=AF.Copy,
                                         accum_out=stat2[:, m:m + 1])
                    sq_sc2 = rot.tile([128, D2], BF, tag="sqd2", bufs=2)
                    nc.scalar.activation(out=sq_sc2[:], in_=zp[:], func=AF.Square,
                                         accum_out=stat2[:, 4 + m:5 + m])

                bi2 = dr.tile([128, 8], F32)
                bo2 = dr.tile([128, 8], F32)
                nc.gpsimd.dma_start(out=bi2[:], in_=stat2[:])
                if sim_mode:
                    nc.gpsimd.dma_start(out=bo2[:], in_=bi2[:])
                else:
                    nc.gpsimd.collective_compute(
                        "AllReduce", ADD, replica_groups=[list(range(NCORES))],
                        ins=[bi2.opt()], outs=[bo2.opt()])
                ast2 = sb.tile([128, 8], F32)
                nc.gpsimd.dma_start(out=ast2[:], in_=bo2[:])

                mu2 = sb.tile([128, 4], F32)
                nc.vector.tensor_scalar(out=mu2[:], in0=ast2[:, 0:4], scalar1=1.0 / B,
                                        scalar2=None, op0=MUL)
                var2 = sb.tile([128, 4], F32)
                nc.vector.tensor_scalar(out=var2[:], in0=ast2[:, 4:8], scalar1=1.0 / B,
                                        scalar2=None, op0=MUL)
                musq2 = sb.tile([128, 4], F32)
                nc.vector.tensor_tensor(out=musq2[:], in0=mu2[:], in1=mu2[:], op=MUL)
                nc.vector.tensor_tensor(out=var2[:], in0=var2[:], in1=musq2[:], op=SUB)
                sd2 = sb.tile([128, 4], F32)
                nc.scalar.activation(out=sd2[:], in_=var2[:], func=AF.Sqrt, bias=eps_bn[:])
                inv2 = sb.tile([128, 4], F32)
                nc.vector.reciprocal(out=inv2[:], in_=sd2[:])
                s2 = sb.tile([128, 4], F32)
                nc.vector.tensor_tensor(out=s2[:], in0=g2[:], in1=inv2[:], op=MUL)
                t2 = sb.tile([128, 4], F32)
                nc.vector.tensor_tensor(out=t2[:], in0=mu2[:], in1=s2[:], op=MUL)
                nc.vector.tensor_tensor(out=t2[:], in0=be2[:], in1=t2[:], op=SUB)

                for m in range(4):  # y2 = s2*z2 + t2 in place (TSP 4x)
                    sl = slice(m * D2, (m + 1) * D2)
                    nc.vector.tensor_scalar(out=z2b[:, sl], in0=z2b[:, sl],
                                            scalar1=s2[:, m:m + 1], scalar2=t2[:, m:m + 1],
                                            op0=MUL, op1=ADD)
                y2 = z2b

                # ---------- Dice 2 (bf16) ----------
                avgp2 = ps.tile([1, BL], F32, tag="cs", bufs=1)
                for m in range(4):
                    nc.tensor.matmul(avgp2[:], lhsT=ones_d2[:], rhs=y2[:, m * D2:(m + 1) * D2],
                                     start=(m == 0), stop=(m == 3))
                avgrow2 = sb.tile([1, BL], BF)
                nc.vector.tensor_copy(out=avgrow2[:], in_=avgp2[:])
                avgb2 = ps.tile([128, BL], F32, tag="bc")
                nc.tensor.matmul(avgb2[:], lhsT=onesrow_bf[:], rhs=avgrow2[:], start=True, stop=True)
                avgb2_sb = sb.tile([128, BL], BF)
                nc.scalar.activation(out=avgb2_sb[:], in_=avgb2[:], func=AF.Copy)

                diff2 = sb.tile([128, 4 * D2], BF)
                for m in range(4):
                    nc.vector.tensor_tensor(out=diff2[:, m * D2:(m + 1) * D2],
                                            in0=y2[:, m * D2:(m + 1) * D2],
                                            in1=avgb2_sb[:], op=SUB)
                varp2 = ps.tile([1, BL], F32, tag="cs", bufs=1)
                for m in range(4):
                    sq_f = rot.tile([128, D2], BF, tag="sqb")
                    nc.vector.tensor_tensor(out=sq_f[:], in0=diff2[:, m * D2:(m + 1) * D2],
                                            in1=diff2[:, m * D2:(m + 1) * D2], op=MUL)
                    nc.tensor.matmul(varp2[:], lhsT=ones_bf[:], rhs=sq_f[:],
                                     start=(m == 0), stop=(m == 3))
                sqrow2 = sb.tile([1, BL], F32)
                nc.scalar.activation(out=sqrow2[:], in_=varp2[:], func=AF.Sqrt, bias=epsd2_row[:])
                rstd2 = sb.tile([1, BL], F32)
                nc.vector.reciprocal(out=rstd2[:], in_=sqrow2[:])
                rstd2b16 = sb.tile([1, BL], BF)
                nc.vector.tensor_copy(out=rstd2b16[:], in_=rstd2[:])
                rstdb2 = ps.tile([128, BL], F32, tag="bc")
                nc.tensor.matmul(rstdb2[:], lhsT=onesrow_bf[:], rhs=rstd2b16[:], start=True, stop=True)
                rstdb2_sb = sb.tile([128, BL], BF)
                nc.scalar.activation(out=rstdb2_sb[:], in_=rstdb2[:], func=AF.Copy)

                for m in range(4):
                    sl = slice(m * D2, (m + 1) * D2)
                    nc.vector.tensor_tensor(out=diff2[:, sl], in0=diff2[:, sl],
                                            in1=rstdb2_sb[:], op=MUL)
                    nc.scalar.activation(out=diff2[:, sl], in_=diff2[:, sl], func=AF.Sigmoid)
                    nc.vector.tensor_scalar(out=diff2[:, sl], in0=diff2[:, sl],
                                            scalar1=oma2[:], scalar2=a2s[:], op0=MUL, op1=ADD)
                    nc.vector.tensor_tensor(out=y2[:, sl], in0=y2[:, sl],
                                            in1=diff2[:, sl], op=MUL)

                # ---------- layer 3: out row = W3^T y2 + b3 ----------
                z3p = ps.tile([1, BL], F32, tag="cs", bufs=1)
                for k in range(4):
                    nc.tensor.matmul(z3p[:], lhsT=w3[:, k:k + 1], rhs=y2[:, k * D2:(k + 1) * D2],
                                     start=(k == 0), stop=(k == 3))
                z3row = sb.tile([1, BL], F32)
                nc.vector.tensor_scalar(out=z3row[:], in0=z3p[:], scalar1=b3s[0:1, 0:1],
                                        scalar2=None, op0=ADD)
                nc.sync.dma_start(out=outd[:], in_=z3row[:])

    nc.compile()
    return nc


def _get_prog():
    global _PROG
    if _PROG is None:
        _PROG = _build()
    return _PROG


def kernel(items, users, history_users, item_table, user_table,
           W1, b1, g1, be1, a1, W2, b2, g2, be2, a2, W3, b3):
    nc = _get_prog()

    items = np.asarray(items).astype(np.int32)
    users = np.asarray(users).astype(np.int32)
    hist = np.asarray(history_users).astype(np.int32)
    item_table = np.ascontiguousarray(np.asarray(item_table, dtype=np.float32))
    user_table = np.ascontiguousarray(np.asarray(user_table, dtype=np.float32))
    W1 = np.asarray(W1, dtype=np.float32)
    W2 = np.asarray(W2, dtype=np.float32)
    W3 = np.asarray(W3, dtype=np.float32)

    # host-side weight reshapes (shared across cores)
    w1sb = W1.reshape(3, 128, D1).transpose(1, 0, 2).reshape(128, 3 * D1)
    w1sb = np.ascontiguousarray(w1sb).astype(ml_dtypes.bfloat16)
    w2sb = W2.reshape(8, 128, D2).transpose(1, 0, 2).reshape(128, 8 * D2)
    w2sb = np.ascontiguousarray(w2sb).astype(ml_dtypes.bfloat16)
    w3sb = np.ascontiguousarray(W3.reshape(4, 128).T).astype(ml_dtypes.bfloat16)
    g1r = np.ascontiguousarray(np.asarray(g1, np.float32).reshape(8, 128).T)
    be1r = np.ascontiguousarray(np.asarray(be1, np.float32).reshape(8, 128).T)
    g2r = np.ascontiguousarray(np.asarray(g2, np.float32).reshape(4, 128).T)
    be2r = np.ascontiguousarray(np.asarray(be2, np.float32).reshape(4, 128).T)
    a1c = np.full((128, 1), np.float32(np.asarray(a1).ravel()[0]), np.float32)
    a2c = np.full((128, 1), np.float32(np.asarray(a2).ravel()[0]), np.float32)
    b3c = np.full((1, 1), np.float32(np.asarray(b3).ravel()[0]), np.float32)

    in_maps = []
    for c in range(NCORES):
        sl = slice(c * BL, (c + 1) * BL)
        idx_hist = hist[sl].reshape(T, 128, H).transpose(1, 0, 2).reshape(128, T * H)
        # cols 0..T-1: users per tile; cols T..2T-1: items per tile
        idx_ui = np.concatenate(
            [users[sl].reshape(T, 128).T, items[sl].reshape(T, 128).T], axis=1)
        in_maps.append({
            "user_table": user_table, "item_table": item_table,
            "idx_hist": np.ascontiguousarray(idx_hist),
            "idx_ui": np.ascontiguousarray(idx_ui),
            "w1sb": w1sb, "w2sb": w2sb, "w3sb": w3sb,
            "g1r": g1r, "be1r": be1r, "g2r": g2r, "be2r": be2r,
            "a1c": a1c, "a2c": a2c, "b3c": b3c,
        })

    res = run_bass_kernel_spmd(nc, in_maps, core_ids=list(range(NCORES)))
    out = np.concatenate(
        [np.asarray(res.results[c]["out"], np.float32).reshape(BL, 1) for c in range(NCORES)],
        axis=0)
    return out
